# revision 1
# baseline (speedup 1.0000x reference)
"""Trainium2 Bass kernel for nn_Attention_71210557768228.

Single-layer non-causal attention with RoPE:
  x:[4,2048,1024] -> qkv (no bias) -> RoPE(q,k) -> softmax(q k^T / 8) v -> proj + bias

Sharding across 8 NeuronCores: core = (batch b in 0..3, head-group g in 0..1).
Each core processes one batch and 8 of the 16 heads end-to-end and produces a
partial projection output [2048, 1024]; the host sums the two head-group
partials per batch and adds the bias.

Per-core layout choices (all matmuls bf16 with fp32 PSUM accumulation):
  - x is fed transposed (xT [1024c, 2048t]) so the contraction dim c sits on
    SBUF partitions for both the q/k projection ([f, t] output) and the v
    projection ([t, dv] output).
  - RoPE: head_dim is permuted host-side (evens then odds inside each
    32-block) so rotate_half becomes a within-quadrant partition rotation,
    done with one DVE stream_shuffle; the sin tile has the rotation signs and
    the swap pre-baked (sinX), cos is plain (cosR). qr = p*cosR + shuffle(p*sinX).
  - scores are computed transposed, sT[j,i] = k_j . q_i, with the two heads of
    a pair row-packed into the 128-deep PE array (d=64 contraction each, at
    partition bases 0 and 64).
  - softmax: no max subtraction (scores*0.125 are small by construction), exp
    on ScalarE straight out of PSUM (scale=0.125 folded into the activation),
    output cast to bf16. The softmax denominator comes for free from a ones
    column appended to V (M=65 attn@v matmul: rows 0..63 = y^T, row 64 = sum).
  - y is normalized with reciprocal * broadcast. The broadcast of 1/rowsum
    across 64 partitions uses two 1-partition DMAs (to quadrant rows 0/32)
    plus a stream_shuffle with an all-zeros mask (GPSIMD partition_broadcast
    ucode is unavailable under this runtime). y is then DMA-repacked so head
    pairs stack into 128 partitions for a K=128 output projection producing
    out[t, o] directly.
  - a single PSUM pool spans all phases (pqk 1 + pv 1 + s0 2 + s64 2 + yu 2
    = 8 banks) and per-pair q/k projections are emitted interleaved with
    attention so the TileScheduler overlaps the phases; the output
    projection reuses the drained phase-A banks.
"""

import os
import sys

import numpy as np
import ml_dtypes

_REPO = "/opt/trn_rl_repo"
if _REPO not in sys.path:
    sys.path.insert(0, _REPO)

import concourse.bass as bass
import concourse.bacc as bacc
import concourse.mybir as mybir
import concourse.tile as tile
from concourse.bass import ts
from concourse.tile import TileContext

F32 = mybir.dt.float32
BF16 = mybir.dt.bfloat16

DIM, H, D = 1024, 16, 64
B, T = 4, 2048
G = 2                 # head groups (cores per batch)
HG = H // G           # heads per group = 8
DV = HG * D           # per-core v width = 512
N_CORES = 8

SWAP16 = [(i + 16) % 32 for i in range(32)]
PROJ_CADENCE = 4

# Schraudolph exp2-in-bf16-bits constants for exp(0.125*s) = 2^(s*k):
# i16 = round(s*k*128 + (127 + C)*128), C tuned for min-max relative ripple
SCHR_K = 0.125 * 1.4426950408889634
SCHR_C = 0.0430
SCHR_A = SCHR_K * 128.0
SCHR_B = (127.0 + SCHR_C) * 128.0


# ---------------------------------------------------------------- host prep

def _perm64():
    perm = np.zeros(64, dtype=np.int64)
    for q in range(2):
        for i in range(16):
            perm[32 * q + i] = 32 * q + 2 * i
            perm[32 * q + 16 + i] = 32 * q + 2 * i + 1
    return perm


def _cos_sin_tiles(freqs):
    """cosR, sinX [128, T] fp32 (rows replicate with period 64)."""
    perm = _perm64()
    cos = np.cos(freqs)            # [T, 64]
    sin = np.sin(freqs)
    cos64 = np.ascontiguousarray(cos[:, perm].T)     # [64, T]
    sinX64 = np.empty_like(cos64)
    for r in range(64):
        q, i = r // 32, r % 32
        sw = 32 * q + ((i + 16) % 32)
        sign = 1.0 if i < 16 else -1.0
        sinX64[r] = sign * sin[:, perm[sw]]
    cosR = np.concatenate([cos64, cos64], axis=0).astype(np.float32)
    sinX = np.concatenate([sinX64, sinX64], axis=0).astype(np.float32)
    return cosR, sinX


# ---------------------------------------------------------------- bass build

def build_nc(pexp_bufs=4, rope_bufs=3, yu_bufs=2, norm_bufs=3, osb_bufs=5,
             dve_exp_jcs=(), fp8_av=False):
    nc = bacc.Bacc("TRN2", target_bir_lowering=False)

    xT_d = nc.dram_tensor("xT", (DIM, T), BF16, kind="ExternalInput")
    wqk_d = nc.dram_tensor("wqkT", (DIM, 2 * DV), BF16, kind="ExternalInput")
    wv_d = nc.dram_tensor("wvT", (DIM, DV), BF16, kind="ExternalInput")
    wp_d = nc.dram_tensor("wpT", (DV, DIM), BF16, kind="ExternalInput")
    cos_d = nc.dram_tensor("cosR", (128, T), F32, kind="ExternalInput")
    sin_d = nc.dram_tensor("sinX", (128, T), F32, kind="ExternalInput")
    out_d = nc.dram_tensor("out_part", (T, DIM), F32, kind="ExternalOutput")

    CT = DIM // 128      # 8 contraction tiles for the projections
    TT = T // 128        # 16 token tiles of 128
    T4 = T // 512        # 4 token slices of 512
    FT = (2 * DV) // 128  # 8 f-tiles (q then k)
    JT = T // 128        # 16 key-token tiles

    with TileContext(nc) as tc:
        with tc.tile_pool(name="const", bufs=1) as cpool:
            # persistent SBUF tensors
            wqk_sb = cpool.tile([128, CT, 2 * DV], BF16)
            wv_sb = cpool.tile([128, CT, DV], BF16)
            wp_sb = cpool.tile([128, DV // 128, DIM], BF16)
            cos_sb = cpool.tile([128, T], F32)
            sin_sb = cpool.tile([128, T], F32)
            qk_sb = cpool.tile([128, FT, T], BF16)
            if fp8_av:
                # [j-tile-pair, ko, head, col]: col 64 = ones (rowsum trick),
                # cols padded to 80 so the DoubleRow weight AP step is 16-aligned
                v_sb = cpool.tile([128, JT // 2, 2, HG, 80], mybir.dt.float8e4)
            else:
                v_sb = cpool.tile([128, JT, HG, D + 1], BF16)
            y2_sb = cpool.tile([128, DV // 128, T], BF16)

            # x and q/k weights first (they gate the first matmuls), then
            # v weights, rope tables, and the projection weights (needed last)

            # ones column for the rowsum trick
            if fp8_av:
                nc.vector.memset(v_sb[:, :, :, :, D], 1.0)
            else:
                nc.vector.memset(v_sb[:, :, :, D], 1.0)
            # seed tile for the reciprocal partition-replication (rows 0/32
            # get the live data; the rest only needs to be initialized once
            # to satisfy read-range tracking)
            rseed = cpool.tile([D, 512], F32)
            nc.vector.memset(rseed[:], 0.0)

            # one PSUM pool shared by all phases so they can overlap:
            # pqk(1) + pv(1) + s0(2) + s64(2) + yu(2) = 8 banks
            with tc.tile_pool(name="pA", bufs=1) as apool, \
                 tc.tile_pool(name="ps", bufs=1, space="PSUM") as psum, \
                 tc.tile_pool(name="rope", bufs=rope_bufs) as rpool, \
                 tc.tile_pool(name="pexp", bufs=pexp_bufs) as pxpool, \
                 tc.tile_pool(name="norm", bufs=norm_bufs) as npool, \
                 tc.tile_pool(name="osb", bufs=osb_bufs) as opool:
                xT_sb = apool.tile([128, CT, T], BF16)
                nc.scalar.dma_start(cos_sb[:], cos_d[:])
                nc.scalar.dma_start(sin_sb[:], sin_d[:])
                for ct in range(CT):
                    nc.sync.dma_start(xT_sb[:, ct, :], xT_d[ts(ct, 128), :])
                    nc.scalar.dma_start(wqk_sb[:, ct, :], wqk_d[ts(ct, 128), :])
                for ct in range(CT):
                    nc.scalar.dma_start(wv_sb[:, ct, :], wv_d[ts(ct, 128), :])
                for dt4 in range(DV // 128):
                    nc.scalar.dma_start(wp_sb[:, dt4, :], wp_d[ts(dt4, 128), :])

                def qk_tile(ft, tq, borrow=None):
                    if borrow is not None:
                        # startup only: the attention score slots are still
                        # unused, borrow them as extra accumulators so the
                        # prefix q/k groups pipeline 4-wide (the first score
                        # matmuls already depend on these tiles' RoPE output,
                        # so the slot hand-off adds no serialization)
                        pqk = psum.tile([128, 2, 512], F32, tag=borrow,
                                        bufs=1, name="pqkb")[:, 0, :]
                    else:
                        pqk = psum.tile([128, 512], F32, tag="pqk", bufs=1, name="pqk")
                    for ct in range(CT):
                        nc.tensor.matmul(
                            pqk,
                            lhsT=wqk_sb[:, ct, ts(ft, 128)],
                            rhs=xT_sb[:, ct, ts(tq, 512)],
                            start=(ct == 0), stop=(ct == CT - 1))
                    tcos = rpool.tile([128, 512], BF16, tag="tcos")
                    tsin = rpool.tile([128, 512], BF16, tag="tsin")
                    tsw = rpool.tile([128, 512], BF16, tag="tsw")
                    nc.vector.tensor_mul(tcos, pqk, cos_sb[:, ts(tq, 512)])
                    nc.vector.tensor_mul(tsin, pqk, sin_sb[:, ts(tq, 512)])
                    nc.vector.stream_shuffle(tsw, tsin, SWAP16)
                    nc.vector.tensor_add(qk_sb[:, ft, ts(tq, 512)], tcos, tsw)

                def v_tile(tt):
                    pv = psum.tile([128, DV], F32, tag="pv", bufs=1, name="pv")
                    for ct in range(CT):
                        nc.tensor.matmul(
                            pv,
                            lhsT=xT_sb[:, ct, ts(tt, 128)],
                            rhs=wv_sb[:, ct, :],
                            start=(ct == 0), stop=(ct == CT - 1))
                    if fp8_av:
                        nc.vector.tensor_copy(
                            v_sb[:, tt // 2, tt % 2, :, 0:D],
                            pv.rearrange("p (h d) -> p h d", h=HG))
                    else:
                        nc.vector.tensor_copy(
                            v_sb[:, tt, :, 0:D],
                            pv.rearrange("p (h d) -> p h d", h=HG))

                # emission order: pair-0's first score chunk needs q-tile
                # (ft0,tq0) and k-tiles (ft4,*); v j-tiles arrive just in time
                # for the attn@v stream; remaining q/k tiles after
                qk_tile(0, 0, borrow="s0")
                qk_tile(4, 0, borrow="s64")
                for tq in range(1, T4):
                    v_tile(2 * (tq - 1))
                    v_tile(2 * (tq - 1) + 1)
                    qk_tile(4, tq, borrow=("s0" if tq == 1 else None))
                for tt in range(6, 16):
                    v_tile(tt)
                for tq in range(1, T4):
                    qk_tile(0, tq)

                # ---- output-projection tile (reuses phase-A psum slots) ----
                proj_done = []

                def proj_tile(tt, on):
                    po = psum.tile([128, 512], F32,
                                   tag=("pqk" if (2 * tt + on) % 2 else "pv"),
                                   bufs=1, name="po")
                    for d4 in range(DV // 128):
                        nc.tensor.matmul(
                            po,
                            lhsT=y2_sb[:, d4, ts(tt, 128)],
                            rhs=wp_sb[:, d4, ts(on, 512)],
                            start=(d4 == 0), stop=(d4 == DV // 128 - 1))
                    ot = opool.tile([128, 512], F32, tag="ot")
                    nc.vector.tensor_copy(ot, po)
                    nc.sync.dma_start(out_d[ts(tt, 128), ts(on, 512)], ot)
                    proj_done.append((tt, on))

                # -------- attention for one head pair (interleaved) --------
                def att_pair(pair):
                    kf, qf = 4 + pair, pair
                    for it in range(T4):
                        yu = {}
                        for half in (0, 64):
                            yu[half] = psum.tile([128, 512], F32, tag="yu", bufs=yu_bufs, name=f"yu{half}")
                        for jc in range(JT // 2):
                            for half in (0, 64):
                                sp = psum.tile(
                                    [128, 2, 512], F32, tag=f"s{half}",
                                    bufs=1, name=f"s{half}")
                                for u in range(2):
                                    jt = 2 * jc + u
                                    nc.tensor.matmul(
                                        sp[:, u, :],
                                        lhsT=qk_sb[half:half + 64, kf, ts(jt, 128)],
                                        rhs=qk_sb[half:half + 64, qf, ts(it, 512)],
                                        start=True, stop=True)
                                if jc in dve_exp_jcs:
                                    # exp2 bit-trick on the (otherwise idle)
                                    # VectorE: bf16 bits of 2^t are about
                                    # round(128*(t + 127 + c)); the constant
                                    # multiplicative bias cancels in the
                                    # softmax ratio, only the mantissa-
                                    # linearization ripple (~±3%) remains,
                                    # which averages out over 2048 keys
                                    tf = rpool.tile([128, 2, 512], F32,
                                                    tag="schr", name="tf")
                                    nc.vector.tensor_scalar(
                                        tf, sp[:],
                                        SCHR_A, SCHR_B,
                                        mybir.AluOpType.mult,
                                        mybir.AluOpType.add)
                                    pexp_i = pxpool.tile(
                                        [128, 2, 512], mybir.dt.int16,
                                        tag=f"px{half}",
                                        bufs=pexp_bufs, name=f"pxi{half}")
                                    nc.vector.tensor_copy(pexp_i, tf)
                                    pexp_c = pexp_i.bitcast(BF16)
                                else:
                                    pexp_c = pxpool.tile(
                                        [128, 2, 512],
                                        mybir.dt.float8e4 if fp8_av else BF16,
                                        tag=f"px{half}",
                                        bufs=pexp_bufs, name=f"px{half}")
                                    nc.scalar.activation(
                                        pexp_c,
                                        sp[:],
                                        mybir.ActivationFunctionType.Exp,
                                        scale=0.125)
                                h = 2 * pair + (half // 64)
                                if fp8_av:
                                    # one DoubleRow matmul per chunk:
                                    # 256-deep contraction (both j-tiles)
                                    nc.tensor.matmul(
                                        yu[half][0:D + 1, :],
                                        lhsT=v_sb[:, jc, :, h, 0:D + 1],
                                        rhs=pexp_c[:],
                                        start=(jc == 0),
                                        stop=(jc == JT // 2 - 1),
                                        perf_mode=mybir.MatmulPerfMode.DoubleRow)
                                else:
                                    for u in range(2):
                                        jt = 2 * jc + u
                                        nc.tensor.matmul(
                                            yu[half][0:D + 1, :],
                                            lhsT=v_sb[:, jt, h, :],
                                            rhs=pexp_c[:, u, :],
                                            start=(jt == 0), stop=(jt == JT - 1))
                            if (pair == HG // 2 - 1 and it >= 1
                                    and jc % PROJ_CADENCE == PROJ_CADENCE - 1):
                                # dribble ready output-projection tiles into
                                # the last pair's PE stream (their y2 token
                                # slices completed in earlier i-iterations)
                                ready = [(tt, on)
                                         for it2 in range(it)
                                         for tt in range(4 * it2, 4 * it2 + 4)
                                         for on in range(DIM // 512)
                                         if (tt, on) not in proj_done]
                                if ready:
                                    proj_tile(*ready[0])
                        for half in (0, 64):
                            h = 2 * pair + (half // 64)
                            rcp = npool.tile([D + 1, 512], F32, tag="rcp")
                            nc.vector.reciprocal(rcp[D:D + 1, :], yu[half][D:D + 1, :])
                            # replicate 1/r to 64 partitions: seed quadrant
                            # rows 0 and 32 via DMA, then an all-zeros
                            # stream_shuffle mask fills each 32-quadrant
                            nc.sync.dma_start(rseed[0:1, :], rcp[D:D + 1, :])
                            nc.sync.dma_start(rseed[32:33, :], rcp[D:D + 1, :])
                            rrep = npool.tile([D, 512], F32, tag="rrep")
                            nc.vector.stream_shuffle(rrep, rseed, [0] * 32)
                            ytmp = npool.tile([D, 512], BF16, tag="ytmp")
                            nc.vector.tensor_mul(ytmp, yu[half][0:D, :], rrep)
                            nc.sync.dma_start(
                                y2_sb[half:half + D, pair, ts(it, 512)], ytmp)

                # interleave: emit each pair's q/k projections right before
                # its attention so the scheduler alternates PE work between
                # attention (ACT-gated) and dense projection fill
                att_pair(0)
                for pr in (1, 2, 3):
                    for tq in range(T4):
                        qk_tile(pr, tq)
                        qk_tile(4 + pr, tq)
                    att_pair(pr)

                # -------------- phase C: remaining projection tiles --------
                for tt in range(TT):
                    for on in range(DIM // 512):
                        if (tt, on) not in proj_done:
                            proj_tile(tt, on)

    nc.finalize()
    return nc


_NC_CACHE = None


def _get_nc():
    global _NC_CACHE
    if _NC_CACHE is None:
        _NC_CACHE = build_nc()
    return _NC_CACHE


# ---------------------------------------------------------------- entry point

def kernel(x, freqs, W_qkv, W_proj, b_proj, _trace=False):
    x = np.asarray(x, dtype=np.float32)
    freqs = np.asarray(freqs, dtype=np.float32)
    W_qkv = np.asarray(W_qkv, dtype=np.float32)
    W_proj = np.asarray(W_proj, dtype=np.float32)
    b_proj = np.asarray(b_proj, dtype=np.float32)

    perm = _perm64()
    cosR, sinX = _cos_sin_tiles(freqs)

    # per-group weight shards
    wqkT = {}
    wvT = {}
    wpT = {}
    for g in range(G):
        rows = []
        for blk in (0, 1):  # q rows then k rows
            for hh in range(HG):
                h = g * HG + hh
                base = blk * DIM + h * D
                rows.append(W_qkv[base + perm])
        wqkT[g] = np.ascontiguousarray(
            np.concatenate(rows, axis=0).T).astype(ml_dtypes.bfloat16)
        wvT[g] = np.ascontiguousarray(
            W_qkv[2 * DIM + g * DV: 2 * DIM + (g + 1) * DV].T
        ).astype(ml_dtypes.bfloat16)
        wpT[g] = np.ascontiguousarray(
            W_proj[:, g * DV:(g + 1) * DV].T).astype(ml_dtypes.bfloat16)

    xT_b = {b: np.ascontiguousarray(x[b].T).astype(ml_dtypes.bfloat16)
            for b in range(B)}
    in_maps = []
    for core in range(N_CORES):
        b, g = core // G, core % G
        in_maps.append({
            "xT": xT_b[b],
            "wqkT": wqkT[g],
            "wvT": wvT[g],
            "wpT": wpT[g],
            "cosR": cosR,
            "sinX": sinX,
        })

    from concourse import bass_utils

    nc = _get_nc()
    res = bass_utils.run_bass_kernel_spmd(
        nc, in_maps, core_ids=list(range(N_CORES)), trace=_trace)

    out = np.zeros((B, T, DIM), dtype=np.float32)
    for core in range(N_CORES):
        b = core // G
        out[b] += res.results[core]["out_part"]
    out += b_proj
    if _trace:
        return out, res
    return out



# revision 2
# speedup vs baseline: 1.1594x; 1.1594x over previous
"""Trainium2 Bass kernel for nn_Attention_71210557768228.

Single-layer non-causal attention with RoPE; 8 cores = (batch, head-group).

Numerics (per-stage, all validated against the fp32 reference):
  - scores: fp8e4 DoubleRow matmuls over packed [32 p, 2 u, T] per-head q/k
    (post-RoPE, one head per quadrant, explicit tile_position).
  - q/k + v projections: fp8 DoubleRow with residual splits (x = x8+xl8,
    W*64 = W8+Wl8; 64x cancels via exp scale and a 64-valued ones column).
    Term count per tile is tunable: the 4 startup tiles run 1-term (so the
    first scores only need x8+W8 loaded), k03 tq1-3 tiles run 2-term
    (x-residual only), the rest 3-term.
  - attn@v and out-projection stay bf16.

Schedule (the cost-model's in-order engines make emission order the
schedule):
  - head-outer, it-inner. Phase A = only the 4 startup q/k tiles + k03 rest.
  - v tiles are emitted JIT inside head 0's first jc sweep, right before the
    attn@v matmuls that consume them.
  - q03 tiles for later its trickle inside head 0's blocks; q47/k47 tiles
    trickle inside heads 1-2; output-projection tiles dribble inside head
    7's blocks (tokens of it-1) and a small phase C.
  - input DMAs: few big rearranged transfers, ordered by first use, on the
    SP queue; late weights on gpsimd; cos/sin on ACT (its seq must stay
    free to dispatch the softmax exp, which paces the kernel).
  - PSUM->SBUF copies (v tiles, proj output) run on the Pool engine.
"""

import os
import sys

import numpy as np
import ml_dtypes

_REPO = "/opt/trn_rl_repo"
if _REPO not in sys.path:
    sys.path.insert(0, _REPO)

import concourse.bass as bass
import concourse.bacc as bacc
import concourse.mybir as mybir
import concourse.tile as tile
from concourse.bass import ts
from concourse.tile import TileContext

F32 = mybir.dt.float32
BF16 = mybir.dt.bfloat16
FP8 = mybir.dt.float8e4

DIM, H, D = 1024, 16, 64
B, T = 4, 2048
G = 2
HG = H // G
DV = HG * D
N_CORES = 8

SWAP16 = [(i + 16) % 32 for i in range(32)]
WS = 64.0

CT2 = DIM // 256
TT = T // 128
T4 = T // 512
JT = T // 128
NFT = 4               # 0=Q(h0-3) 1=K(h0-3) 2=Q(h4-7) 3=K(h4-7)

DR = mybir.MatmulPerfMode.DoubleRow


def _dmap():
    dm = np.zeros((32, 2), dtype=np.int64)
    for u in range(2):
        for i in range(32):
            j = u * 16 + (i % 16)
            elem = i // 16
            dm[i, u] = 2 * j + elem
    return dm


def _cos_sin_tiles(freqs):
    dm = _dmap()
    cos = np.cos(freqs)
    sin = np.sin(freqs)
    cosP = np.zeros((128, 2, T), dtype=np.float32)
    sinS = np.zeros((128, 2, T), dtype=np.float32)
    for i in range(32):
        for u in range(2):
            d = dm[i, u]
            isw = (i + 16) % 32
            d_dest = dm[isw, u]
            sign_dest = -1.0 if isw < 16 else 1.0
            for q in range(4):
                cosP[32 * q + i, u] = cos[:, d]
                sinS[32 * q + i, u] = sign_dest * sin[:, d_dest]
    return cosP.astype(ml_dtypes.bfloat16), sinS.astype(ml_dtypes.bfloat16)


# ---------------------------------------------------------------- bass build

def build_nc(pexp_bufs=6, rope_bufs=3, yu_bufs=2, norm_bufs=3, osb_bufs=5,
             s_bufs=2, pool_copies=False, startup_terms=1, k123_terms=2,
             wide_c=True, split_tables=True, proj2=False, defer0=False,
             yus_bufs=9):
    nc = bacc.Bacc("TRN2", target_bir_lowering=False)

    x8_d = nc.dram_tensor("x8", (DIM, T), FP8, kind="ExternalInput")
    xl_d = nc.dram_tensor("xl8", (DIM, T), FP8, kind="ExternalInput")
    wqk8_d = nc.dram_tensor("wqk8", (DIM, NFT * 2 * 128), FP8, kind="ExternalInput")
    wqkl_d = nc.dram_tensor("wqkl8", (DIM, NFT * 2 * 128), FP8, kind="ExternalInput")
    wv2_d = nc.dram_tensor("wv28", (DIM, 2 * DV), FP8, kind="ExternalInput")
    wp_d = nc.dram_tensor("wpT", (DV, DIM), BF16, kind="ExternalInput")
    trig_d = nc.dram_tensor("trig", (128, 2 * 2 * T), BF16, kind="ExternalInput")
    out_d = nc.dram_tensor("out_part", (T, DIM), F32, kind="ExternalOutput")

    def dsrc(dten, cols):
        return dten[:, :].rearrange("(c p) t -> p c t", p=128)[:, :, cols]

    with TileContext(nc) as tc:
        with tc.tile_pool(name="const", bufs=1) as cpool:
            x8_sb = cpool.tile([128, CT2, 2, T], FP8)
            xl_sb = cpool.tile([128, CT2, 2, T], FP8)
            wqk8_sb = cpool.tile([128, CT2, 2, NFT, 2, 128], FP8)
            wqkl_sb = cpool.tile([128, CT2, 2, NFT, 2, 128], FP8)
            wv2_sb = cpool.tile([128, CT2, 2, 2, DV], FP8)
            wv8_sb = wv2_sb[:, :, :, 0]
            wvl_sb = wv2_sb[:, :, :, 1]
            wp_sb = cpool.tile([128, DV // 128, DIM], BF16)
            trig_sb = cpool.tile([128, 2, 2, T], BF16)
            cos_sb = trig_sb[:, 0]
            sin_sb = trig_sb[:, 1]
            qk8 = cpool.tile([128, NFT, 2, T], FP8)
            v_sb = cpool.tile([128, JT, HG, D + 1], BF16)
            y2_sb = cpool.tile([128, DV // 128, T], BF16)

            nc.vector.memset(v_sb[:, :, :, D], WS)
            rseed = cpool.tile([D, 512], F32)
            nc.vector.memset(rseed[:], 0.0)

            with tc.tile_pool(name="pA", bufs=1) as apool, \
                 tc.tile_pool(name="ps", bufs=1, space="PSUM") as psum, \
                 tc.tile_pool(name="rope", bufs=rope_bufs) as rpool, \
                 tc.tile_pool(name="pexp", bufs=pexp_bufs) as pxpool, \
                 tc.tile_pool(name="norm", bufs=norm_bufs) as npool, \
                 tc.tile_pool(name="osb", bufs=osb_bufs) as opool:

                # ---- input DMAs: one ordered stream on sync (transfers
                # serialize on a single DMA track; order = arrival schedule)
                def tsrc(cols):
                    return trig_d[:, :].rearrange(
                        "p (a u t) -> p a u t", a=2, u=2)[:, :, :, cols]

                nc.sync.dma_start(
                    wqk8_sb[:, :, :, 0:2, :, :], dsrc(wqk8_d, slice(0, 512)))
                nc.sync.dma_start(x8_sb[:, :, :, 0:512],
                                  dsrc(x8_d, slice(0, 512)))
                nc.sync.dma_start(trig_sb[:, :, :, 0:512], tsrc(slice(0, 512)))
                nc.sync.dma_start(x8_sb[:, :, :, 512:1024],
                                  dsrc(x8_d, slice(512, 1024)))
                nc.sync.dma_start(
                    wqkl_sb[:, :, :, 0:2, :, :], dsrc(wqkl_d, slice(0, 512)))
                nc.sync.dma_start(trig_sb[:, :, :, 512:1024],
                                  tsrc(slice(512, 1024)))
                nc.sync.dma_start(
                    wv2_sb[:, :, :, :, :].rearrange("p c u a d -> p (c u) (a d)"),
                    wv2_d[:, :].rearrange("(c p) d -> p c d", p=128))
                nc.sync.dma_start(xl_sb[:, :, :, 0:512],
                                  dsrc(xl_d, slice(0, 512)))
                nc.sync.dma_start(x8_sb[:, :, :, 1024:1536],
                                  dsrc(x8_d, slice(1024, 1536)))
                nc.sync.dma_start(trig_sb[:, :, :, 1024:1536],
                                  tsrc(slice(1024, 1536)))
                nc.sync.dma_start(x8_sb[:, :, :, 1536:T],
                                  dsrc(x8_d, slice(1536, T)))
                nc.sync.dma_start(trig_sb[:, :, :, 1536:T],
                                  tsrc(slice(1536, T)))
                nc.sync.dma_start(xl_sb[:, :, :, 512:1024],
                                  dsrc(xl_d, slice(512, 1024)))
                nc.sync.dma_start(xl_sb[:, :, :, 1024:T],
                                  dsrc(xl_d, slice(1024, T)))
                nc.sync.dma_start(
                    wqk8_sb[:, :, :, 2:4, :, :], dsrc(wqk8_d, slice(512, 1024)))
                nc.sync.dma_start(
                    wqkl_sb[:, :, :, 2:4, :, :], dsrc(wqkl_d, slice(512, 1024)))
                nc.gpsimd.dma_start(
                    wp_sb[:, :, :],
                    wp_d[:, :].rearrange("(c p) o -> p c o", p=128))

                # v copies on DVE (early; Pool is busy with weight DMAs),
                # proj-output copies on Pool (late; Pool is idle by then)
                vcpy = nc.vector
                ocpy = nc.gpsimd if pool_copies else nc.vector

                def qk_tile(ft, u, tq, borrow=None, terms=3):
                    if borrow == "s":
                        pqk = psum.tile([128, 2, 512], F32, tag="s",
                                        bufs=s_bufs, name="pqkb")[:, 0, :]
                    elif borrow == "yu":
                        pqk = psum.tile([128, 512], F32, tag="yu",
                                        bufs=yu_bufs, name="pqky")
                    else:
                        pqk = psum.tile([128, 512], F32, tag="pqk", bufs=1,
                                        name="pqk")
                    if terms == 2:
                        # W-residual: keeps xl off the startup critical path
                        tl = [(wqk8_sb, x8_sb), (wqkl_sb, x8_sb)]
                    else:
                        tl = [(wqk8_sb, x8_sb), (wqk8_sb, xl_sb),
                              (wqkl_sb, x8_sb)][:terms]
                    n = len(tl) * CT2
                    i = 0
                    for wsb, xsb in tl:
                        for ct2 in range(CT2):
                            nc.tensor.matmul(
                                pqk,
                                lhsT=wsb[:, ct2, :, ft, u, :],
                                rhs=xsb[:, ct2, :, ts(tq, 512)],
                                start=(i == 0), stop=(i == n - 1),
                                perf_mode=DR)
                            i += 1
                    tcos = rpool.tile([128, 512], BF16, tag="tcos")
                    tsin = rpool.tile([128, 512], BF16, tag="tsin")
                    tsw = rpool.tile([128, 512], BF16, tag="tsw")
                    nc.vector.tensor_mul(tcos, pqk, cos_sb[:, u, ts(tq, 512)])
                    nc.vector.tensor_mul(tsin, pqk, sin_sb[:, u, ts(tq, 512)])
                    nc.vector.stream_shuffle(tsw, tsin, SWAP16)
                    nc.vector.tensor_add(qk8[:, ft, u, ts(tq, 512)], tcos, tsw)

                def v_tile(tt):
                    pv = psum.tile([128, DV], F32, tag="pv", bufs=1, name="pv")
                    tl = [(x8_sb, wv8_sb), (x8_sb, wvl_sb), (xl_sb, wv8_sb)]
                    n = len(tl) * CT2
                    i = 0
                    for xsb, wsb in tl:
                        for ct2 in range(CT2):
                            nc.tensor.matmul(
                                pv,
                                lhsT=xsb[:, ct2, :, ts(tt, 128)],
                                rhs=wsb[:, ct2, :, :],
                                start=(i == 0), stop=(i == n - 1),
                                perf_mode=DR)
                            i += 1
                    vcpy.tensor_copy(
                        v_sb[:, tt, :, 0:D],
                        pv.rearrange("p (h d) -> p h d", h=HG))

                proj_done = []

                def proj_tile(tt, on, wide=False):
                    k = len(proj_done)
                    if wide:
                        tg = ("pqk", "pv", "s", "yu")[k % 4]
                        if tg == "s":
                            po = psum.tile([128, 2, 512], F32, tag="s",
                                           bufs=s_bufs, name="pow")[:, 0, :]
                        elif tg == "yu":
                            po = psum.tile([128, 512], F32, tag="yu",
                                           bufs=yu_bufs, name="poy")
                        else:
                            po = psum.tile([128, 512], F32, tag=tg, bufs=1,
                                           name="po")
                    else:
                        po = psum.tile([128, 512], F32,
                                       tag=("pqk" if (2 * tt + on) % 2 else "pv"),
                                       bufs=1, name="po")
                    for d4 in range(DV // 128):
                        nc.tensor.matmul(
                            po,
                            lhsT=y2_sb[:, d4, ts(tt, 128)],
                            rhs=wp_sb[:, d4, ts(on, 512)],
                            start=(d4 == 0), stop=(d4 == DV // 128 - 1))
                    ot = opool.tile([128, 512], F32, tag="ot")
                    (ocpy if k % 2 else nc.vector).tensor_copy(ot, po)
                    outq = [nc.sync, nc.gpsimd][k % 2]
                    outq.dma_start(out_d[ts(tt, 128), ts(on, 512)], ot)
                    proj_done.append((tt, on))

                # fill work emitted inside attention streams, keyed (it, h, jc)
                FILL = {}

                def add_fill(it, h, jc, fn, *args, **kw):
                    FILL.setdefault((it, h, jc), []).append((fn, args, kw))

                # it0 block h0: k03 tq1-3 JIT (needed by jc2/4/6)
                for i, tq in enumerate((1, 2, 3)):
                    add_fill(0, 0, i, qk_tile, 1, 0, tq, terms=k123_terms)
                    add_fill(0, 0, i, qk_tile, 1, 1, tq, terms=k123_terms)
                # it0: q47-tq0 in block 2, k47 in blocks 3-4 (late so their
                # weight DMAs have landed), q tiles for it1 in blocks 5-7
                it0 = [(2, 1, (2, 0, 0), 3), (2, 4, (2, 1, 0), 3),
                       (2, 6, (3, 0, 0), 2), (3, 1, (3, 1, 0), 2),
                       (3, 3, (3, 0, 1), 2), (3, 5, (3, 1, 1), 2),
                       (3, 7, (3, 0, 2), 2), (4, 0, (3, 1, 2), 2),
                       (4, 2, (3, 0, 3), 2), (4, 3, (3, 1, 3), 2),
                       (5, 2, (0, 0, 1), 3), (5, 5, (0, 1, 1), 3),
                       (6, 2, (2, 0, 1), 3), (7, 2, (2, 1, 1), 3)]
                for h, jc, (ft, u, tq), terms in it0:
                    add_fill(0, h, jc, qk_tile, ft, u, tq, terms=terms)
                # it1/it2: q tiles for the next it
                for it, (ft, u) in ((1, (0, 0)), (1, (0, 1)),
                                    (1, (2, 0)), (1, (2, 1)),
                                    (2, (0, 0)), (2, (0, 1)),
                                    (2, (2, 0)), (2, (2, 1))):
                    h = {(0, 0): 0, (0, 1): 2, (2, 0): 4, (2, 1): 6}[(ft, u)]
                    add_fill(it, h, 3, qk_tile, ft, u, it + 1, terms=3)

                yus_p = {}

                def norm_step(h, it, ysrc):
                    rcp = npool.tile([D + 1, 512], F32, tag="rcp")
                    nc.vector.reciprocal(rcp[D:D + 1, :], ysrc[D:D + 1, :])
                    nc.sync.dma_start(rseed[0:1, :], rcp[D:D + 1, :])
                    nc.sync.dma_start(rseed[32:33, :], rcp[D:D + 1, :])
                    rrep = npool.tile([D, 512], F32, tag="rrep")
                    nc.vector.stream_shuffle(rrep, rseed, [0] * 32)
                    ytmp = npool.tile([D, 512], BF16, tag="ytmp")
                    nc.vector.tensor_mul(ytmp, ysrc[0:D, :], rrep)
                    nc.sync.dma_start(
                        y2_sb[64 * (h % 2):64 * (h % 2) + D, h // 2,
                              ts(it, 512)], ytmp)

                def att_chunk(h, it, jc, yu):
                    ftq, ftk, base = 2 * (h // 4), 2 * (h // 4) + 1, 32 * (h % 4)
                    sp = psum.tile([128, 2, 512], F32, tag="s",
                                   bufs=s_bufs, name="s")
                    for u2 in range(2):
                        jt = 2 * jc + u2
                        nc.tensor.matmul(
                            sp[:, u2, :],
                            lhsT=qk8[base:base + 32, ftk, :, ts(jt, 128)],
                            rhs=qk8[base:base + 32, ftq, :, ts(it, 512)],
                            start=True, stop=True,
                            perf_mode=DR, tile_position=(base, 0))
                    pexp = pxpool.tile([128, 2, 512], BF16, tag="px",
                                       bufs=pexp_bufs, name="px")
                    nc.scalar.activation(
                        pexp, sp[:],
                        mybir.ActivationFunctionType.Exp,
                        scale=0.125 / (WS * WS))
                    for u2 in range(2):
                        jt = 2 * jc + u2
                        nc.tensor.matmul(
                            yu[0:D + 1, :],
                            lhsT=v_sb[:, jt, h, :],
                            rhs=pexp[:, u2, :],
                            start=(jt == 0), stop=(jt == JT - 1))
                    for fn, args, kw in FILL.get((it, h, jc), ()):
                        fn(*args, **kw)
                    if it >= 1 and (jc == 6 or (proj2 and jc == 3)):
                        ready = [(tt, on)
                                 for tt in range(4 * (it - 1), 4 * it)
                                 for on in range(DIM // 512)
                                 if (tt, on) not in proj_done]
                        if ready:
                            proj_tile(*ready[0])

                def att_block(h, it):
                    yu = psum.tile([128, 512], F32, tag="yu",
                                   bufs=yu_bufs, name="yu")
                    for jc in range(JT // 2):
                        att_chunk(h, it, jc, yu)
                    norm_step(h, it, yu)

                def att_pair(ha, hb, it):
                    # two heads interleaved at chunk granularity: the second
                    # head's chunks need no new data, so they keep the exp
                    # stream busy while the first head's fills/inputs land
                    yu0 = psum.tile([128, 512], F32, tag="yu",
                                    bufs=yu_bufs, name="yu0")
                    yu1 = psum.tile([128, 512], F32, tag="yu",
                                    bufs=yu_bufs, name="yu1")
                    for jc in range(JT // 2):
                        if ha == 0 and it == 0:
                            v_tile(2 * jc)
                            v_tile(2 * jc + 1)
                        att_chunk(ha, it, jc, yu0)
                        att_chunk(hb, it, jc, yu1)
                    norm_step(ha, it, yu0)
                    norm_step(hb, it, yu1)

                # ---- phase A: minimal startup ----
                qk_tile(0, 0, 0, borrow="s", terms=startup_terms)
                qk_tile(0, 1, 0, borrow="s", terms=startup_terms)
                qk_tile(1, 0, 0, borrow="yu", terms=startup_terms)
                qk_tile(1, 1, 0, borrow="yu", terms=startup_terms)

                for it in range(T4):
                    if it == 0:
                        att_pair(0, 1, it)
                        for h in range(2, HG):
                            att_block(h, it)
                    else:
                        for h in range(HG):
                            att_block(h, it)

                for tt in range(TT):
                    for on in range(DIM // 512):
                        if (tt, on) not in proj_done:
                            proj_tile(tt, on, wide=wide_c)

    nc.finalize()
    return nc


_NC_CACHE = None


def _get_nc():
    global _NC_CACHE
    if _NC_CACHE is None:
        _NC_CACHE = build_nc()
    return _NC_CACHE


# ---------------------------------------------------------------- entry point

def kernel(x, freqs, W_qkv, W_proj, b_proj, _trace=False):
    x = np.asarray(x, dtype=np.float32)
    freqs = np.asarray(freqs, dtype=np.float32)
    W_qkv = np.asarray(W_qkv, dtype=np.float32)
    W_proj = np.asarray(W_proj, dtype=np.float32)
    b_proj = np.asarray(b_proj, dtype=np.float32)

    dm = _dmap()
    cosP, sinS = _cos_sin_tiles(freqs)

    def split8(a):
        hi = a.astype(ml_dtypes.float8_e4m3fn)
        lo = (a - hi.astype(np.float32)).astype(ml_dtypes.float8_e4m3fn)
        return hi, lo

    wqk8T, wqklT, wv8T, wvlT, wpT = {}, {}, {}, {}, {}
    for g in range(G):
        cols = []
        for ft in range(NFT):
            qk = ft % 2
            for u in range(2):
                for hq in range(4):
                    h = g * HG + 4 * (ft // 2) + hq
                    base = qk * DIM + h * D
                    cols.append(W_qkv[base + dm[:, u]])
        wqkT_full = np.ascontiguousarray(
            np.concatenate(cols, axis=0).T).astype(np.float32) * WS
        wqk8T[g], wqklT[g] = split8(wqkT_full)
        wvT_full = np.ascontiguousarray(
            W_qkv[2 * DIM + g * DV: 2 * DIM + (g + 1) * DV].T
        ).astype(np.float32) * WS
        wv8T[g], wvlT[g] = split8(wvT_full)
        wpT[g] = np.ascontiguousarray(
            W_proj[:, g * DV:(g + 1) * DV].T).astype(ml_dtypes.bfloat16)

    x8_b, xl_b = {}, {}
    for b in range(B):
        x8_b[b], xl_b[b] = split8(np.ascontiguousarray(x[b].T))

    trig = np.concatenate(
        [cosP.reshape(128, 2 * T), sinS.reshape(128, 2 * T)],
        axis=1)  # [128, 2*2*T] bf16: cos block then sin block
    wv2 = {g: np.concatenate([wv8T[g], wvlT[g]], axis=1) for g in range(G)}
    in_maps = []
    for core in range(N_CORES):
        b, g = core // G, core % G
        in_maps.append({
            "x8": x8_b[b],
            "xl8": xl_b[b],
            "wqk8": wqk8T[g],
            "wqkl8": wqklT[g],
            "wv28": wv2[g],
            "wpT": wpT[g],
            "trig": trig,
        })

    from concourse import bass_utils

    nc = _get_nc()
    res = bass_utils.run_bass_kernel_spmd(
        nc, in_maps, core_ids=list(range(N_CORES)), trace=_trace)

    out = np.zeros((B, T, DIM), dtype=np.float32)
    for core in range(N_CORES):
        b = core // G
        out[b] += res.results[core]["out_part"]
    out += b_proj
    if _trace:
        return out, res
    return out
